# revision 1
# baseline (speedup 1.0000x reference)
"""Trainium2 Bass kernel for nn_ContinuousEmbedding (masked matmul + bias).

Computes out = x @ (weights * mask) + bias, reshaped to [B, in_size, out_size],
where mask zeroes each input feature's own [out_size]-wide diagonal block.

Strategy: tensor-parallel across the 8 NeuronCores by splitting the
in_size*out_size (=16384) output columns into 8 shards of 2048 columns.
Each core gets the full x (as x^T for the matmul's stationary operand),
its weight-column shard (mask is constant — folded into the weights on the
host), and its bias shard. Per core: out_shard = x @ W_shard + bias_shard
via 128x128 fp32 matmuls accumulating K=256 as 2 PSUM passes; bias-add is
fused into the PSUM->SBUF eviction on the vector engine.
"""

import numpy as np

B = 4096
IN_SIZE = 256
OUT_SIZE = 64
IO = IN_SIZE * OUT_SIZE          # 16384
N_CORES = 8
N_SHARD = IO // N_CORES          # 2048 output columns per core
P = 128                          # SBUF partitions
KO = IN_SIZE // P                # 2 contraction sub-tiles
N_TILE = 512                     # matmul moving free dim (fp32 max)
M_TILES = B // P                 # 32 output row tiles
N_TILES = N_SHARD // N_TILE      # 4 column tiles per core

MATMUL_MODE = "fp32r"            # "fp32" | "fp32r" | "fp32x3"

_CACHE: dict = {}


def _build_program(mode=None):
    mode = mode or MATMUL_MODE
    import concourse.mybir as mybir
    import concourse.tile as tile
    import concourse.bass as bass
    from concourse import bacc

    nsplit = 2 if mode == "fp32x3" else 1
    nc = bacc.Bacc(
        "TRN2", target_bir_lowering=False, debug=False, num_devices=N_CORES
    )
    # fp32r modes: operands are pre-rounded on the host to fp32r's 12
    # significand bits (round-to-nearest-even), so the DRAM tensors are
    # declared float32r and plain (non-casting) HWDGE DMAs load them.
    in_dt = mybir.dt.float32 if mode == "fp32" else mybir.dt.float32r
    xt = nc.dram_tensor(
        "xt", [nsplit, IN_SIZE, B], in_dt, kind="ExternalInput"
    ).ap()
    w = nc.dram_tensor(
        "w", [nsplit, IN_SIZE, N_SHARD], in_dt, kind="ExternalInput"
    ).ap()
    bias = nc.dram_tensor(
        "bias", [N_SHARD], mybir.dt.float32, kind="ExternalInput"
    ).ap()
    out = nc.dram_tensor(
        "out", [B, N_SHARD], mybir.dt.float32, kind="ExternalOutput"
    ).ap()

    with tile.TileContext(nc) as tc:
        with tc.tile_pool(name="const", bufs=1) as const, \
             tc.tile_pool(name="psum", bufs=2, space="PSUM") as psum_pool, \
             tc.tile_pool(name="outp", bufs=4) as outp:
            mm_dt = (mybir.dt.float32 if mode == "fp32"
                     else mybir.dt.float32r)
            w_sb = const.tile([P, nsplit, KO, N_SHARD], mm_dt)
            xt_sb = const.tile([P, nsplit, KO, B], mm_dt)
            bias_sb = const.tile([P, N_SHARD], mybir.dt.float32)

            # Whole-tensor DMAs keep the per-partition packets large
            # (8-16KB) — fragmented loads pay per-packet overhead.
            ld_eng = nc.sync
            w_src = w.rearrange("s (ko p) n -> p s ko n", p=P)
            ld_eng.dma_start(out=w_sb[:], in_=w_src[:])
            # x^T load: first chunk smaller so m-tile 0 starts sooner.
            xt_src = xt.rearrange("s (ko p) m -> p s ko m", p=P)
            for lo, hi in [(0, 1024), (1024, B)]:
                sl = slice(lo, hi)
                ld_eng.dma_start(out=xt_sb[:, :, :, sl], in_=xt_src[:, :, :, sl])
            # bias [N_SHARD] broadcast across all 128 partitions (stride-0
            # DRAM read).
            bias_bcast = bass.AP(
                tensor=bias.tensor,
                offset=bias.offset,
                ap=[[0, P]] + list(bias.ap),
            )
            ld_eng.dma_start(out=bias_sb[:], in_=bias_bcast)

            # (x_split, w_split) matmul terms: plain modes use (0,0);
            # fp32x3 adds the hi/lo cross terms (lo@hi, hi@lo).
            terms = [(0, 0)] if nsplit == 1 else [(0, 0), (1, 0), (0, 1)]
            for m in range(M_TILES):
                out_sb = outp.tile([P, N_SHARD], mybir.dt.float32)
                for n in range(N_TILES):
                    ns = slice(n * N_TILE, (n + 1) * N_TILE)
                    ps = psum_pool.tile([P, N_TILE], mybir.dt.float32,
                                        name=f"ps{n}", tag=f"ps{n}")
                    nmm = KO * len(terms)
                    for i, (k, (xi, wi)) in enumerate(
                        (k, t) for k in range(KO) for t in terms
                    ):
                        nc.tensor.matmul(
                            ps[:],
                            lhsT=xt_sb[:, xi, k, m * P:(m + 1) * P],
                            rhs=w_sb[:, wi, k, ns],
                            start=(i == 0),
                            stop=(i == nmm - 1),
                        )
                    nc.vector.tensor_add(out_sb[:, ns], ps[:], bias_sb[:, ns])
                nc.sync.dma_start(out=out[m * P:(m + 1) * P, :], in_=out_sb[:])

    nc.compile()
    return nc


def _get_program(mode=None):
    mode = mode or MATMUL_MODE
    if mode not in _CACHE:
        _CACHE[mode] = _build_program(mode)
    return _CACHE[mode]


def _round12(a):
    """Round fp32 to fp32r's 12 significand bits (round-to-nearest-even)."""
    u = a.view(np.uint32)
    r = (u + np.uint32(0x7FF) + ((u >> np.uint32(12)) & np.uint32(1)))
    return (r & np.uint32(0xFFFFF000)).view(np.float32)


def _hi_lo(a):
    hi = _round12(a)
    return np.stack([hi, _round12(a - hi)], axis=0)


def _shard_inputs(x, weights, bias, mode=None):
    mode = mode or MATMUL_MODE
    # Fold the constant block-diagonal mask into the weights on the host.
    col_block = np.arange(IO, dtype=np.int64) // OUT_SIZE
    mask = (col_block[None, :] != np.arange(IN_SIZE)[:, None])
    wm = weights * mask.astype(weights.dtype)
    xt = np.ascontiguousarray(x.T)
    if mode == "fp32x3":
        xt_in = _hi_lo(xt)
    elif mode == "fp32r":
        xt_in = _round12(xt)[None]
    else:
        xt_in = xt[None]
    in_maps = []
    for c in range(N_CORES):
        sl = slice(c * N_SHARD, (c + 1) * N_SHARD)
        w_shard = np.ascontiguousarray(wm[:, sl])
        if mode == "fp32x3":
            w_in = _hi_lo(w_shard)
        elif mode == "fp32r":
            w_in = _round12(w_shard)[None]
        else:
            w_in = w_shard[None]
        in_maps.append({
            "xt": xt_in,
            "w": np.ascontiguousarray(w_in),
            "bias": np.ascontiguousarray(bias[sl]),
        })
    return in_maps


def run_sharded(in_maps, mode=None, **kwargs):
    """Run the SPMD program on cores 0-7. kwargs forwarded (e.g. trace)."""
    from concourse.bass_utils import run_bass_kernel_spmd

    nc = _get_program(mode)
    return run_bass_kernel_spmd(
        nc, in_maps, core_ids=list(range(N_CORES)), **kwargs
    )


def kernel(x: np.ndarray, weights: np.ndarray, bias: np.ndarray) -> np.ndarray:
    x = np.asarray(x, dtype=np.float32)
    weights = np.asarray(weights, dtype=np.float32)
    bias = np.asarray(bias, dtype=np.float32)
    in_maps = _shard_inputs(x, weights, bias)
    res = run_sharded(in_maps)
    full = np.concatenate([res.results[c]["out"] for c in range(N_CORES)], axis=1)
    return full.reshape(B, IN_SIZE, OUT_SIZE)



# revision 2
# speedup vs baseline: 1.7044x; 1.7044x over previous
"""Trainium2 Bass kernel for nn_ContinuousEmbedding (masked matmul + bias).

Computes out = x @ (weights * mask) + bias, reshaped to [B, in_size, out_size],
where mask zeroes each input feature's own [out_size]-wide diagonal block.

Strategy: tensor-parallel across the 8 NeuronCores by splitting the
in_size*out_size (=16384) output columns into 8 shards of 2048 columns.
The rel-err budget (2e-2) is large, so all matmul I/O is bf16: inputs are
cast on the host, the PE runs bf16 at full rate, and the output shard is
stored to HBM as bf16 (halving the dominant store traffic) then upcast on
the host.

Compute orientation is TRANSPOSED vs the torch view: each core computes
out_t[col, batch] = W_shard.T-contracted with x, i.e. matmul with
lhsT = W[k, col_block] (stationary) and rhs = x^T[k, batch] (moving).
That puts the io-columns on PSUM partitions, so the bias becomes a
per-partition scalar — eviction is a 1-op fused add+cast via
tensor_scalar (DVE) / activation-Identity (ACT), alternating between the
two engines so eviction keeps up with the PE. The host transposes the
gathered [2048, 4096] shards back to [B, io].

Mask is constant — folded into the weights on the host.
"""

import numpy as np

B = 4096
IN_SIZE = 256
OUT_SIZE = 64
IO = IN_SIZE * OUT_SIZE          # 16384
N_CORES = 8
N_SHARD = IO // N_CORES          # 2048 output columns per core
P = 128                          # SBUF/PSUM partitions
KO = IN_SIZE // P                # 2 contraction sub-tiles
M_BLOCKS = N_SHARD // P          # 16 col-blocks per core
N_TILE = 512                     # matmul moving free dim (fp32 PSUM bank)
G_TILE = 1024                    # eviction group width (2 PSUM banks)
G_PER_M = B // G_TILE            # 4 groups per col-block
PSUM_BUFS = 4                    # 4 x 2 banks = all 8 PSUM banks

_CACHE: dict = {}


def _build_program():
    import concourse.mybir as mybir
    import concourse.tile as tile
    import concourse.bass as bass
    from concourse import bacc

    nc = bacc.Bacc(
        "TRN2", target_bir_lowering=False, debug=False, num_devices=N_CORES
    )
    bf16 = mybir.dt.bfloat16
    f32 = mybir.dt.float32
    xt = nc.dram_tensor("xt", [IN_SIZE, B], bf16, kind="ExternalInput").ap()
    w = nc.dram_tensor("w", [IN_SIZE, N_SHARD], bf16, kind="ExternalInput").ap()
    # bias pre-swizzled on host: bias_sw[p, m] = bias_shard[m*128 + p]
    bias = nc.dram_tensor("bias", [P, M_BLOCKS], f32, kind="ExternalInput").ap()
    # transposed output shard: out_t[col, batch]
    out = nc.dram_tensor("out", [N_SHARD, B], bf16, kind="ExternalOutput").ap()

    with tile.TileContext(nc) as tc:
        with tc.tile_pool(name="const", bufs=1) as const, \
             tc.tile_pool(name="psum", bufs=PSUM_BUFS, space="PSUM") as psum_pool, \
             tc.tile_pool(name="outp", bufs=3) as outp:
            w_sb = const.tile([P, KO, N_SHARD], bf16)
            xt_sb = const.tile([P, KO, B], bf16)
            bias_sb = const.tile([P, M_BLOCKS], f32)

            ld = nc.sync
            # Weights first (the first LDWEIGHTS gates everything).
            w_src = w.rearrange("(ko p) n -> p ko n", p=P)
            ld.dma_start(out=w_sb[:], in_=w_src[:])
            # x^T in k-half x batch-half chunks so col-block 0 can start
            # after ~2 MiB landed instead of 3.
            xt_src = xt.rearrange("(ko p) n -> p ko n", p=P)
            for lo, hi in [(0, B // 2), (B // 2, B)]:
                for k in range(KO):
                    ld.dma_start(
                        out=xt_sb[:, k, lo:hi], in_=xt_src[:, k, lo:hi]
                    )
            ld.dma_start(out=bias_sb[:], in_=bias[:])

            # Warm the ACT activation-table before the first real eviction.
            warm = const.tile([1, 1], f32)
            nc.vector.memset(warm, 0.0)
            nc.scalar.add(warm[:], warm[:], bias_sb[0:1, 0:1])

            for m in range(M_BLOCKS):
                ms = slice(m * P, (m + 1) * P)
                out_sb = outp.tile([P, B], bf16, name=f"osb{m}", tag="osb")
                for g in range(G_PER_M):
                    ps = psum_pool.tile([P, G_TILE], f32, name=f"ps{m}_{g}",
                                        tag="ps")
                    for k in range(KO):
                        for s in range(G_TILE // N_TILE):
                            ns = slice(g * G_TILE + s * N_TILE,
                                       g * G_TILE + (s + 1) * N_TILE)
                            nc.tensor.matmul(
                                ps[:, s * N_TILE:(s + 1) * N_TILE],
                                lhsT=w_sb[:, k, ms],
                                rhs=xt_sb[:, k, ns],
                                start=(k == 0),
                                stop=(k == KO - 1),
                            )
                    gs = slice(g * G_TILE, (g + 1) * G_TILE)
                    if (m * G_PER_M + g) % 2 == 0:
                        nc.vector.tensor_scalar_add(
                            out_sb[:, gs], ps[:], bias_sb[:, m:m + 1]
                        )
                    else:
                        nc.scalar.add(out_sb[:, gs], ps[:], bias_sb[:, m:m + 1])
                    # Store per batch-half to smooth the DMA stream and
                    # shorten the tail after the last eviction.
                    if g % 2 == 1:
                        hs = slice((g - 1) * G_TILE, (g + 1) * G_TILE)
                        ld.dma_start(out=out[ms, hs], in_=out_sb[:, hs])

    nc.compile()
    return nc


def _get_program():
    if "prog" not in _CACHE:
        _CACHE["prog"] = _build_program()
    return _CACHE["prog"]


def _shard_inputs(x, weights, bias):
    import ml_dtypes

    bf16 = ml_dtypes.bfloat16
    # Fold the constant block-diagonal mask into the weights on the host.
    col_block = np.arange(IO, dtype=np.int64) // OUT_SIZE
    mask = (col_block[None, :] != np.arange(IN_SIZE)[:, None])
    wm = (weights * mask.astype(weights.dtype)).astype(bf16)
    xt = np.ascontiguousarray(x.T.astype(bf16))
    in_maps = []
    for c in range(N_CORES):
        sl = slice(c * N_SHARD, (c + 1) * N_SHARD)
        in_maps.append({
            "xt": xt,
            "w": np.ascontiguousarray(wm[:, sl]),
            "bias": np.ascontiguousarray(
                bias[sl].astype(np.float32).reshape(M_BLOCKS, P).T
            ),
        })
    return in_maps


def run_sharded(in_maps, **kwargs):
    """Run the SPMD program on cores 0-7. kwargs forwarded (e.g. trace)."""
    from concourse.bass_utils import run_bass_kernel_spmd

    nc = _get_program()
    return run_bass_kernel_spmd(
        nc, in_maps, core_ids=list(range(N_CORES)), **kwargs
    )


def kernel(x: np.ndarray, weights: np.ndarray, bias: np.ndarray) -> np.ndarray:
    x = np.asarray(x, dtype=np.float32)
    weights = np.asarray(weights, dtype=np.float32)
    bias = np.asarray(bias, dtype=np.float32)
    in_maps = _shard_inputs(x, weights, bias)
    res = run_sharded(in_maps)
    full = np.empty((B, IO), dtype=np.float32)
    for c in range(N_CORES):
        sl = slice(c * N_SHARD, (c + 1) * N_SHARD)
        full[:, sl] = np.asarray(res.results[c]["out"]).astype(np.float32).T
    return full.reshape(B, IN_SIZE, OUT_SIZE)
